# revision 1
# baseline (speedup 1.0000x reference)
"""Trainium2 Bass kernel for NaiveRNN.

Reference computation:
    xi = x @ W_i2h + b_i2h                    # [B, L, D_h]
    h_{t+1} = tanh(xi_t + h_t @ W_h2h + b_h2h)  # L sequential steps
    out = h_L @ W_out + b_out                 # [B, D_out]

Sharding: data-parallel over batch B=128 across 8 cores (16 rows each).
Weights replicated. No cross-core communication.

Per-core kernel structure:
  Phase 1 (fp32r matmuls): xi' = x_loc @ W_i2h + (b_i2h + b_h2h), written
      to DRAM scratch [L, B_loc, D_h] so each step's slice is contiguous.
      x tiles are PE-transposed (f32 DMA transpose unsupported).
  Phase 2: 512 recurrence steps, bf16 W/h (full 1 cyc/col PE rate; fp32r
      measured ~1.5 cyc/col). State kept transposed: hT [128, 8, 16] bf16,
      which feeds matmul lhsT directly. Each step, per 512-column half:
        z_psum = I16 @ xi_t  (fp32r identity matmul injects xi on the PE,
                              keeping DVE off the critical path)
        z_psum += hT.T @ W_h2h   (8 bf16 matmuls, W as moving operand)
        h_new = tanh(z_psum)     (ACT, PSUM -> SBUF bf16)
      then one bf16 DMA-transpose (16x128 XBAR) per half turns h_new
      [16, 512] into hT [128, 4, 16] for the next step - no PE
      transposes, no DVE work on the critical path.
  Phase 3: out = h_L @ W_out + b_out (bf16 + fp32r bias matmul).
"""

import numpy as np

B, L, D_IN, D_H, D_OUT = 128, 512, 512, 1024, 512
NCORES = 8
BL = B // NCORES            # 16 local batch rows
KI = D_IN // 128            # 4 k-chunks for input proj
KH = D_H // 128             # 8 k-chunks for recurrence
ROW_TILES = (BL * L) // 128  # 64 row tiles in phase 1
LW = L // 128               # l-windows per batch row group (4)


def build_nc(l_steps=L):
    import concourse.bass as bass
    import concourse.mybir as mybir
    from concourse import bacc
    from concourse.tile import TileContext
    from concourse.masks import make_identity

    dt = mybir.dt
    f32, f32r, bf16 = dt.float32, dt.float32r, dt.bfloat16
    AF = mybir.ActivationFunctionType
    ALU = mybir.AluOpType

    nc = bacc.Bacc(
        "TRN2", target_bir_lowering=False, debug=False, num_devices=NCORES
    )
    x = nc.dram_tensor("x", [BL * L, D_IN], f32, kind="ExternalInput")
    W_i2h = nc.dram_tensor("W_i2h", [D_IN, D_H], f32, kind="ExternalInput")
    b_i2h = nc.dram_tensor("b_i2h", [D_H], f32, kind="ExternalInput")
    W_h2h = nc.dram_tensor("W_h2h", [D_H, D_H], f32, kind="ExternalInput")
    b_h2h = nc.dram_tensor("b_h2h", [D_H], f32, kind="ExternalInput")
    W_out = nc.dram_tensor("W_out", [D_H, D_OUT], f32, kind="ExternalInput")
    b_out = nc.dram_tensor("b_out", [D_OUT], f32, kind="ExternalInput")
    out = nc.dram_tensor("out", [BL, D_OUT], f32, kind="ExternalOutput")
    # NB: keep xi_dram plain f32 — float32r-typed DMAs on the sync
    # (HWDGE) ring corrupt subsequent DMA-transposes (HW-reproduced).
    # The per-step load casts f32 -> f32r on the gpsimd (SWDGE) ring.
    xi_dram = nc.dram_tensor(
        "xi_scratch", [L, BL, D_H], f32, kind="Internal"
    )

    with TileContext(nc) as tc:
        with tc.tile_pool(name="const", bufs=1) as cpool:
            # Persistent weights/constants in SBUF. gpsimd DMA casts
            # f32 -> f32r / bf16 during the load.
            whh = cpool.tile([128, KH, D_H], bf16, tag="whh")
            wi2h = cpool.tile([128, KI, D_H], f32r, tag="wi2h")
            wout = cpool.tile([128, KH, D_OUT], bf16, tag="wout")
            nc.gpsimd.dma_start(
                whh[:], W_h2h.ap().rearrange("(ko p) n -> p ko n", p=128)
            )
            nc.gpsimd.dma_start(
                wi2h[:], W_i2h.ap().rearrange("(ko p) n -> p ko n", p=128)
            )
            nc.gpsimd.dma_start(
                wout[:], W_out.ap().rearrange("(ko p) n -> p ko n", p=128)
            )
            ident = cpool.tile([128, 128], f32, tag="ident")
            make_identity(nc, ident[:])
            i16r = cpool.tile([BL, BL], f32r, tag="i16r")
            nc.vector.tensor_copy(i16r[:], ident[:BL, :BL])
            ones_f = cpool.tile([1, 128], f32, tag="ones_f")
            nc.gpsimd.memset(ones_f[:], 1.0)
            ones_row = cpool.tile([1, 128], f32r, tag="ones")
            nc.vector.tensor_copy(ones_row[:], ones_f[:])
            bi = cpool.tile([1, D_H], f32, tag="bi")
            nc.sync.dma_start(bi[:], b_i2h.ap().unsqueeze(0))
            bh = cpool.tile([1, D_H], f32, tag="bh")
            nc.sync.dma_start(bh[:], b_h2h.ap().unsqueeze(0))
            bcomb = cpool.tile([1, D_H], f32r, tag="bcomb")
            nc.vector.tensor_add(bcomb[:], bi[:], bh[:])
            bo_f = cpool.tile([1, D_OUT], f32, tag="bo_f")
            nc.sync.dma_start(bo_f[:], b_out.ap().unsqueeze(0))
            bo = cpool.tile([1, D_OUT], f32r, tag="bo")
            nc.vector.tensor_copy(bo[:], bo_f[:])

            # ---------------- Phase 1: xi' = x @ W_i2h + bcomb ----------------
            with (
                tc.tile_pool(name="p1", bufs=3) as p1pool,
                tc.tile_pool(name="p1ps_t", bufs=3, space="PSUM") as p1ps_t,
                tc.tile_pool(name="p1ps_z", bufs=2, space="PSUM") as p1ps_z,
            ):
                for r in range(ROW_TILES):
                    b_idx = r // LW
                    lw = r % LW
                    xrow = p1pool.tile([128, D_IN], f32, tag="xrow")
                    nc.sync.dma_start(
                        xrow[:], x[128 * r : 128 * r + 128, :]
                    )
                    xT = p1pool.tile([128, KI, 128], f32r, tag="xT")
                    for j in range(KI):
                        xTps = p1ps_t.tile([128, 128], f32, tag="xTps")
                        nc.tensor.transpose(
                            xTps[:], xrow[:, 128 * j : 128 * j + 128], ident[:]
                        )
                        nc.vector.tensor_copy(xT[:, j, :], xTps[:])
                    xi_sb = p1pool.tile([128, D_H], f32, tag="xi_sb")
                    for h in range(2):
                        ns = slice(512 * h, 512 * h + 512)
                        zp = p1ps_z.tile([128, 512], f32, tag="zp1")
                        for k in range(KI):
                            nc.tensor.matmul(
                                zp[:],
                                xT[:, k, :],
                                wi2h[:, k, ns],
                                start=(k == 0),
                                stop=False,
                            )
                        nc.tensor.matmul(
                            zp[:],
                            ones_row[:, :128],
                            bcomb[:, ns],
                            start=False,
                            stop=True,
                        )
                        nc.vector.tensor_copy(xi_sb[:, ns], zp[:])
                    nc.sync.dma_start(
                        xi_dram[128 * lw : 128 * lw + 128, b_idx, :], xi_sb[:]
                    )

            # ---------------- Phase 2: recurrence ----------------
            with (
                tc.tile_pool(name="p2", bufs=1) as p2pool,
                tc.tile_pool(name="p2xi", bufs=6) as xipool,
                tc.tile_pool(name="p2h", bufs=2) as hpool,
                tc.tile_pool(name="p2ps_z", bufs=2, space="PSUM") as p2ps_z,
            ):
                # state: transposed h, as 2 half-tiles (lo: dh<512, hi:
                # dh>=512) per ping-pong buffer. Ping-pong A/B so the
                # DMA-transpose write never aliases chunks the current
                # step's matmuls still read; the lo/hi split gives Tile
                # per-half dependencies so next-step matmuls k<4 start
                # while the hi-half transpose is still in flight.
                hT_a = [
                    p2pool.tile(
                        [128, KH // 2, BL], bf16, tag=f"hT_a{i}",
                        name=f"hT_a{i}",
                    )
                    for i in range(2)
                ]
                hT_b = [
                    p2pool.tile(
                        [128, KH // 2, BL], bf16, tag=f"hT_b{i}",
                        name=f"hT_b{i}",
                    )
                    for i in range(2)
                ]
                zeros_f = p2pool.tile([128, KH * BL // 2], f32, tag="zeros_f")
                nc.gpsimd.memset(zeros_f[:], 0.0)
                for i in range(2):
                    nc.vector.tensor_copy(
                        hT_a[i][:].rearrange("p a b -> p (a b)"), zeros_f[:]
                    )

                for t in range(l_steps):
                    hT_cur, hT_nxt = (
                        (hT_a, hT_b) if t % 2 == 0 else (hT_b, hT_a)
                    )
                    xi_t = xipool.tile([BL, D_H], f32r, tag="xi_t")
                    nc.gpsimd.dma_start(xi_t[:], xi_dram[t, :, :])
                    zp = p2ps_z.tile([BL, D_H], f32, tag="zp2")
                    h_new = hpool.tile([BL, D_H], bf16, tag="h_new")
                    for h in range(2):
                        ns = slice(512 * h, 512 * h + 512)
                        # inject xi_t via identity matmul (fp32r, exact)
                        nc.tensor.matmul(
                            zp[:, ns],
                            i16r[:],
                            xi_t[:, ns],
                            start=True,
                            stop=False,
                        )
                        for k in range(KH):
                            nc.tensor.matmul(
                                zp[:, ns],
                                hT_cur[k // 4][:, k % 4, :],
                                whh[:, k, ns],
                                start=False,
                                stop=(k == KH - 1),
                            )
                        nc.scalar.activation(h_new[:, ns], zp[:, ns], AF.Tanh)
                        # transpose this half into the other state buffer.
                        # Issued from the scalar (ACT) HWDGE ring: it queues
                        # right behind the tanh on the producer's own
                        # sequencer, minimizing cross-engine latency.
                        nc.scalar.dma_start(
                            hT_nxt[h][:],
                            h_new[:, ns],
                            transpose=True,
                        )

                # ---------------- Phase 3: head ----------------
                hT_fin = hT_a if l_steps % 2 == 0 else hT_b
                zp3 = p2ps_z.tile([BL, D_OUT], f32, tag="zp3")
                nc.tensor.matmul(
                    zp3[:],
                    ones_row[:, :BL],
                    bo[:],
                    start=True,
                    stop=False,
                )
                for k in range(KH):
                    nc.tensor.matmul(
                        zp3[:],
                        hT_fin[k // 4][:, k % 4, :],
                        wout[:, k, :],
                        start=False,
                        stop=(k == KH - 1),
                    )
                out_sb = p2pool.tile([BL, D_OUT], f32, tag="out_sb")
                nc.vector.tensor_copy(out_sb[:], zp3[:])
                nc.sync.dma_start(out.ap(), out_sb[:])

    nc.compile()
    return nc


_CACHE = {}


def _get_nc(l_steps=L):
    if l_steps not in _CACHE:
        _CACHE[l_steps] = build_nc(l_steps)
    return _CACHE[l_steps]


def run(inputs, l_steps=L, trace=False, tmpdir=None):
    from concourse.bass_utils import run_bass_kernel_spmd

    nc = _get_nc(l_steps)
    x = np.asarray(inputs["x"], np.float32).reshape(B, L, D_IN)
    shared = {
        k: np.ascontiguousarray(np.asarray(inputs[k], np.float32))
        for k in ("W_i2h", "b_i2h", "W_h2h", "b_h2h", "W_out", "b_out")
    }
    in_maps = []
    for c in range(NCORES):
        m = dict(shared)
        m["x"] = np.ascontiguousarray(
            x[c * BL : (c + 1) * BL].reshape(BL * L, D_IN)
        )
        in_maps.append(m)
    res = run_bass_kernel_spmd(
        nc,
        in_maps,
        core_ids=list(range(NCORES)),
        trace=trace,
        tmpdir=tmpdir,
    )
    out = np.concatenate([r["out"] for r in res.results], axis=0)
    return out, res


def kernel(**inputs) -> np.ndarray:
    out, _ = run(inputs)
    return out



# revision 7
# speedup vs baseline: 2.9721x; 2.9721x over previous
"""Trainium2 Bass kernel for NaiveRNN.

Reference computation:
    xi = x @ W_i2h + b_i2h                      # [B, L, D_h]
    h_{t+1} = tanh(xi_t + h_t @ W_h2h + b_h2h)  # L sequential steps
    out = h_L @ W_out + b_out                   # [B, D_out]

Sharding: data-parallel over batch B=128 across 8 cores (16 rows each).
Weights replicated. No cross-core communication.

Per-core kernel design (all bf16 on the PE, f32 PSUM accumulation):

  Phase 1: xi' = x @ W_i2h + (b_i2h + b_h2h), written to DRAM scratch
      [L, B_loc, D_h] bf16. x tiles are cast-loaded bf16 (SWDGE) and
      transposed by HWDGE DMA-transpose (off the critical path, 3-deep
      pipelined). Bias injected via a zero-padded K=128 ones matmul so
      every matmul keeps the (128,128) array mode (no drains).

  Phase 2: 512 recurrence steps. The batch (16 rows) uses only 16 of the
      PE's 128 stationary columns, so the array is run in 128x32
      column-tiling mode: 4 concurrent matmuls per k-round, each
      streaming a different quarter of W_h2h's columns (N=256) ->
      ~4x less PE streaming time per step than a single-tile schedule.

      Column->tile assignment is interleaved at 32-column granularity
      (tile j takes dh columns with (dh//32)%4 == j, realized purely by
      strided access patterns on the W/xi streams: tiles laid out as
      [p, k, cc, j, w] are bit-identical to the plain layout). With this
      assignment the tanh output h_new[32j+b, 32cc+w] = h[b, 128cc+32j+w]
      turns into the next step's stationary operand hT[p, 32k+v] =
      h[v, 128k+p] via a single in-place 32x32 block transpose -- which
      is exactly what the DVE's stream-transpose instruction does.
      So the per-step recurrent path is:
        PE: 4 xi-inject matmuls (zero-padded identity, K=128) +
            8 rounds x 4 col-tiled matmuls (N=256)     -> PSUM [128,256]
        ACT: tanh PSUM -> SBUF bf16 [128,256]          (~0.4us)
        DVE: 2 stream-transposes [128,128]             (~0.3us each)
      No DMA and no PE-transpose on the critical path (the baseline lost
      ~5us/step to DMA-transpose latency + HAM cold-clock oscillation).

  Phase 3: out = h_L @ W_out + b_out (bf16 matmuls, f32 out).
"""

import numpy as np

B, L, D_IN, D_H, D_OUT = 128, 512, 512, 1024, 512
NCORES = 8
BL = B // NCORES            # 16 local batch rows
KI = D_IN // 128            # 4 k-chunks for input proj
KH = D_H // 128             # 8 k-chunks for recurrence
ROW_TILES = (BL * L) // 128  # 64 row tiles in phase 1
LW = L // 128               # l-windows per batch row (4)
NT = 4                      # column tiles in phase 2


def build_nc(l_steps=L):
    import concourse.bass as bass
    import concourse.mybir as mybir
    from concourse import bacc
    from concourse.tile import TileContext
    from concourse.masks import make_identity

    dt = mybir.dt
    f32, bf16 = dt.float32, dt.bfloat16
    AF = mybir.ActivationFunctionType

    nc = bacc.Bacc(
        "TRN2", target_bir_lowering=False, debug=False, num_devices=NCORES
    )
    x = nc.dram_tensor("x", [BL * L, D_IN], f32, kind="ExternalInput")
    W_i2h = nc.dram_tensor("W_i2h", [D_IN, D_H], f32, kind="ExternalInput")
    b_i2h = nc.dram_tensor("b_i2h", [D_H], f32, kind="ExternalInput")
    W_h2h = nc.dram_tensor("W_h2h", [D_H, D_H], f32, kind="ExternalInput")
    b_h2h = nc.dram_tensor("b_h2h", [D_H], f32, kind="ExternalInput")
    W_out = nc.dram_tensor("W_out", [D_H, D_OUT], f32, kind="ExternalInput")
    b_out = nc.dram_tensor("b_out", [D_OUT], f32, kind="ExternalInput")
    out = nc.dram_tensor("out", [BL, D_OUT], f32, kind="ExternalOutput")
    xi_dram = nc.dram_tensor("xi_scratch", [L, BL, D_H], bf16, kind="Internal")

    with TileContext(nc) as tc:
        with tc.tile_pool(name="const", bufs=1) as cpool:
            # Persistent weights/constants in SBUF; SWDGE casts f32->bf16.
            # whh laid out [p, k, cc, j, w] == plain [p, k, n] bit-for-bit;
            # slicing [:, k, :, j, :] yields the interleaved N=256 stream
            # for column-tile j.
            whh = cpool.tile([128, KH, KH, NT, 32], bf16, tag="whh")
            wi2h = cpool.tile([128, KI, D_H], bf16, tag="wi2h")
            wout = cpool.tile([128, KH, D_OUT], bf16, tag="wout")
            nc.gpsimd.dma_start(
                whh[:].rearrange("p k a j w -> p k (a j w)"),
                W_h2h.ap().rearrange("(k p) n -> p k n", p=128),
            )
            nc.gpsimd.dma_start(
                wi2h[:], W_i2h.ap().rearrange("(k p) n -> p k n", p=128)
            )
            nc.gpsimd.dma_start(
                wout[:], W_out.ap().rearrange("(k p) n -> p k n", p=128)
            )
            # K=128 zero-padded identity: rows 0-15 = I16, rest 0.
            ipad = cpool.tile([128, BL], bf16, tag="ipad")
            nc.gpsimd.memset(ipad[:], 0.0)
            make_identity(nc, ipad[:BL, :BL], nomemset=True)
            # K=128 zero-padded ones row (for bias injection matmuls).
            ones_pad = cpool.tile([128, 128], bf16, tag="ones_pad")
            nc.gpsimd.memset(ones_pad[:], 0.0)
            nc.gpsimd.memset(ones_pad[:1, :], 1.0)
            # combined bias (b_i2h + b_h2h), zero-padded to K=128 rows.
            bi = cpool.tile([1, D_H], f32, tag="bi")
            nc.sync.dma_start(bi[:], b_i2h.ap().unsqueeze(0))
            bh = cpool.tile([1, D_H], f32, tag="bh")
            nc.sync.dma_start(bh[:], b_h2h.ap().unsqueeze(0))
            bcomb = cpool.tile([128, D_H], bf16, tag="bcomb")
            nc.gpsimd.memset(bcomb[:], 0.0)
            nc.vector.tensor_add(bcomb[:1, :], bi[:], bh[:])
            # phase-3 bias (K=1 matmul; single mode switch is fine there)
            bo_f = cpool.tile([1, D_OUT], f32, tag="bo_f")
            nc.sync.dma_start(bo_f[:], b_out.ap().unsqueeze(0))
            bo = cpool.tile([1, D_OUT], bf16, tag="bo")
            nc.vector.tensor_copy(bo[:], bo_f[:])
            ones_row = cpool.tile([1, 128], bf16, tag="ones")
            nc.gpsimd.memset(ones_row[:], 1.0)

            # ---------------- Phase 1: xi' = x @ W_i2h + bcomb ----------------
            with (
                tc.tile_pool(name="p1", bufs=3) as p1pool,
                tc.tile_pool(name="p1ps", bufs=2, space="PSUM") as p1ps,
            ):
                for r in range(ROW_TILES):
                    b_idx = r // LW
                    lw = r % LW
                    xrow = p1pool.tile([128, D_IN], bf16, tag="xrow")
                    nc.gpsimd.dma_start(
                        xrow[:], x[128 * r : 128 * r + 128, :]
                    )
                    xT = p1pool.tile([128, KI, 128], bf16, tag="xT")
                    nc.sync.dma_start(xT[:], xrow[:], transpose=True)
                    xi_sb = p1pool.tile([128, D_H], bf16, tag="xi_sb")
                    for h in range(2):
                        ns = slice(512 * h, 512 * h + 512)
                        zp = p1ps.tile([128, 512], f32, tag="zp1")
                        for k in range(KI):
                            nc.tensor.matmul(
                                zp[:],
                                xT[:, k, :],
                                wi2h[:, k, ns],
                                start=(k == 0),
                                stop=False,
                            )
                        nc.tensor.matmul(
                            zp[:],
                            ones_pad[:],
                            bcomb[:, ns],
                            start=False,
                            stop=True,
                        )
                        nc.scalar.activation(xi_sb[:, ns], zp[:], AF.Copy)
                    nc.scalar.dma_start(
                        xi_dram[128 * lw : 128 * lw + 128, b_idx, :], xi_sb[:]
                    )

            # ---------------- Phase 2: recurrence ----------------
            with (
                tc.tile_pool(name="p2h", bufs=2) as hpool,
                tc.tile_pool(name="p2ps", bufs=2, space="PSUM") as p2ps,
            ):
                # xi ring: [p, cc, j, w] == plain [p, n]; rows 16-127 are
                # zeroed once (finite junk x zero ipad rows = exact 0).
                xi_bufs = [
                    cpool.tile(
                        [128, KH, NT, 32], bf16, tag=f"xi{i}", name=f"xi{i}"
                    )
                    for i in range(3)
                ]
                for t_ in xi_bufs:
                    nc.gpsimd.memset(t_[:], 0.0)
                # transposed-state ping-pong, split lo/hi (k<4 / k>=4) so
                # next-step matmuls can start as soon as the lo half lands.
                hT = [
                    [
                        cpool.tile(
                            [128, 4, 32],
                            bf16,
                            tag=f"hT{p_}{h_}",
                            name=f"hT{p_}{h_}",
                        )
                        for h_ in range(2)
                    ]
                    for p_ in range(2)
                ]
                for h_ in range(2):
                    nc.gpsimd.memset(hT[0][h_][:], 0.0)
                # PSUM ping-pong tiles; zero once so the never-written junk
                # rows (16-31 of each 32-row group) stay finite for tanh.
                zeros_sb = cpool.tile([128, 256], f32, tag="zeros_sb")
                nc.gpsimd.memset(zeros_sb[:], 0.0)
                zps = [
                    p2ps.tile([128, 256], f32, tag="zp2", name=f"zp2_{i}")
                    for i in range(2)
                ]
                for z_ in zps:
                    nc.vector.tensor_copy(z_[:], zeros_sb[:])

                for t in range(l_steps):
                    hT_cur = hT[t % 2]
                    hT_nxt = hT[(t + 1) % 2]
                    xi_t = xi_bufs[t % 3]
                    nc.gpsimd.dma_start(
                        xi_t[:BL].rearrange("p a j w -> p (a j w)"),
                        xi_dram[t, :, :],
                    )
                    zp = zps[t % 2]
                    h_new = hpool.tile([128, KH, 32], bf16, tag="h_new")
                    # xi inject: 4 col-tiled K=128 identity matmuls
                    for j in range(NT):
                        nc.tensor.matmul(
                            zp[32 * j : 32 * j + BL, :],
                            ipad[:],
                            xi_t[:, :, j, :],
                            start=True,
                            stop=False,
                            tile_position=(0, 32 * j),
                        )
                    # 8 k-rounds x 4 concurrent col-tiles
                    for k in range(KH):
                        lhsT = hT_cur[k // 4][:, k % 4, :BL]
                        for j in range(NT):
                            nc.tensor.matmul(
                                zp[32 * j : 32 * j + BL, :],
                                lhsT,
                                whh[:, k, :, j, :],
                                start=False,
                                stop=(k == KH - 1),
                                tile_position=(0, 32 * j),
                            )
                    nc.scalar.activation(
                        h_new[:].rearrange("p a w -> p (a w)"), zp[:], AF.Tanh
                    )
                    # in-place 32x32 block transposes: h_new -> next hT
                    for h_ in range(2):
                        nc.vector.transpose(
                            hT_nxt[h_][:].rearrange("p a w -> p (a w)"),
                            h_new[:, 4 * h_ : 4 * h_ + 4, :].rearrange(
                                "p a w -> p (a w)"
                            ),
                        )

                # ---------------- Phase 3: head ----------------
                hT_fin = hT[l_steps % 2]
                zp3 = p2ps.tile([BL, D_OUT], f32, tag="zp3")
                nc.tensor.matmul(
                    zp3[:],
                    ones_row[:, :BL],
                    bo[:],
                    start=True,
                    stop=False,
                )
                for k in range(KH):
                    nc.tensor.matmul(
                        zp3[:],
                        hT_fin[k // 4][:, k % 4, :BL],
                        wout[:, k, :],
                        start=False,
                        stop=(k == KH - 1),
                    )
                out_sb = cpool.tile([BL, D_OUT], f32, tag="out_sb")
                nc.vector.tensor_copy(out_sb[:], zp3[:])
                nc.sync.dma_start(out.ap(), out_sb[:])

    nc.compile()
    return nc


_CACHE = {}


def _get_nc(l_steps=L):
    if l_steps not in _CACHE:
        _CACHE[l_steps] = build_nc(l_steps)
    return _CACHE[l_steps]


def run(inputs, l_steps=L, trace=False, tmpdir=None):
    from concourse.bass_utils import run_bass_kernel_spmd

    nc = _get_nc(l_steps)
    x = np.asarray(inputs["x"], np.float32).reshape(B, L, D_IN)
    shared = {
        k: np.ascontiguousarray(np.asarray(inputs[k], np.float32))
        for k in ("W_i2h", "b_i2h", "W_h2h", "b_h2h", "W_out", "b_out")
    }
    in_maps = []
    for c in range(NCORES):
        m = dict(shared)
        m["x"] = np.ascontiguousarray(
            x[c * BL : (c + 1) * BL].reshape(BL * L, D_IN)
        )
        in_maps.append(m)
    res = run_bass_kernel_spmd(
        nc,
        in_maps,
        core_ids=list(range(NCORES)),
        trace=trace,
        tmpdir=tmpdir,
    )
    out = np.concatenate([r["out"] for r in res.results], axis=0)
    return out, res


def kernel(**inputs) -> np.ndarray:
    out, _ = run(inputs)
    return out


# revision 10
# speedup vs baseline: 3.2005x; 1.0768x over previous
"""Trainium2 Bass kernel for NaiveRNN.

Reference computation:
    xi = x @ W_i2h + b_i2h                      # [B, L, D_h]
    h_{t+1} = tanh(xi_t + h_t @ W_h2h + b_h2h)  # L sequential steps
    out = h_L @ W_out + b_out                   # [B, D_out]

Sharding: data-parallel over batch B=128 across 8 cores (16 rows each).
Weights replicated. No cross-core communication.

Per-core kernel design (all bf16 on the PE, f32 PSUM accumulation):

  Phase 1: xi' = x @ W_i2h + (b_i2h + b_h2h), written to DRAM scratch
      [L, B_loc, D_h] bf16. x tiles are cast-loaded bf16 (SWDGE) and
      transposed by HWDGE DMA-transpose (off the critical path, 3-deep
      pipelined). Bias injected via a zero-padded K=128 ones matmul so
      every matmul keeps the (128,128) array mode (no drains).

  Phase 2: 512 recurrence steps. The batch (16 rows) uses only 16 of the
      PE's 128 stationary columns, so the array is run in 128x32
      column-tiling mode: 4 concurrent matmuls per k-round, each
      streaming a different quarter of W_h2h's columns (N=256) ->
      ~4x less PE streaming time per step than a single-tile schedule.

      Column->tile assignment is interleaved at 32-column granularity
      (tile j takes dh columns with (dh//32)%4 == j, realized purely by
      strided access patterns on the W/xi streams: tiles laid out as
      [p, k, cc, j, w] are bit-identical to the plain layout). With this
      assignment the tanh output h_new[32j+b, 32cc+w] = h[b, 128cc+32j+w]
      turns into the next step's stationary operand hT[p, 32k+v] =
      h[v, 128k+p] via a single in-place 32x32 block transpose -- which
      is exactly what the DVE's stream-transpose instruction does.
      So the per-step recurrent path is:
        PE: 4 xi-inject matmuls (zero-padded identity, K=128) +
            8 rounds x 4 col-tiled matmuls (N=256)     -> PSUM [128,256]
        ACT: tanh PSUM -> SBUF bf16 [128,256]          (~0.4us)
        DVE: 2 stream-transposes [128,128]             (~0.3us each)
      No DMA and no PE-transpose on the critical path (the baseline lost
      ~5us/step to DMA-transpose latency + HAM cold-clock oscillation).

  Phase 3: out = h_L @ W_out + b_out (bf16 matmuls, f32 out).
"""

import numpy as np

B, L, D_IN, D_H, D_OUT = 128, 512, 512, 1024, 512
NCORES = 8
BL = B // NCORES            # 16 local batch rows
KI = D_IN // 128            # 4 k-chunks for input proj
KH = D_H // 128             # 8 k-chunks for recurrence
ROW_TILES = (BL * L) // 128  # 64 row tiles in phase 1
LW = L // 128               # l-windows per batch row (4)
NT = 4                      # column tiles in phase 2


def build_nc(l_steps=L):
    import concourse.bass as bass
    import concourse.mybir as mybir
    from concourse import bacc
    from concourse.tile import TileContext
    from concourse.masks import make_identity

    dt = mybir.dt
    f32, bf16 = dt.float32, dt.bfloat16
    AF = mybir.ActivationFunctionType

    nc = bacc.Bacc(
        "TRN2", target_bir_lowering=False, debug=False, num_devices=NCORES
    )
    x = nc.dram_tensor("x", [BL * L, D_IN], f32, kind="ExternalInput")
    W_i2h = nc.dram_tensor("W_i2h", [D_IN, D_H], f32, kind="ExternalInput")
    b_i2h = nc.dram_tensor("b_i2h", [D_H], f32, kind="ExternalInput")
    W_h2h = nc.dram_tensor("W_h2h", [D_H, D_H], f32, kind="ExternalInput")
    b_h2h = nc.dram_tensor("b_h2h", [D_H], f32, kind="ExternalInput")
    W_out = nc.dram_tensor("W_out", [D_H, D_OUT], f32, kind="ExternalInput")
    b_out = nc.dram_tensor("b_out", [D_OUT], f32, kind="ExternalInput")
    out = nc.dram_tensor("out", [BL, D_OUT], f32, kind="ExternalOutput")
    xi_dram = nc.dram_tensor("xi_scratch", [L, BL, D_H], bf16, kind="Internal")

    with TileContext(nc) as tc:
        with tc.tile_pool(name="const", bufs=1) as cpool:
            # Persistent weights/constants in SBUF; SWDGE casts f32->bf16.
            # whh laid out [p, k, cc, j, w] == plain [p, k, n] bit-for-bit;
            # slicing [:, k, :, j, :] yields the interleaved N=256 stream
            # for column-tile j.
            whh = cpool.tile([128, KH, KH, NT, 32], bf16, tag="whh")
            wi2h = cpool.tile([128, KI, D_H], bf16, tag="wi2h")
            wout = cpool.tile([128, KH, D_OUT], bf16, tag="wout")
            nc.gpsimd.dma_start(
                whh[:].rearrange("p k a j w -> p k (a j w)"),
                W_h2h.ap().rearrange("(k p) n -> p k n", p=128),
            )
            nc.gpsimd.dma_start(
                wi2h[:], W_i2h.ap().rearrange("(k p) n -> p k n", p=128)
            )
            nc.gpsimd.dma_start(
                wout[:], W_out.ap().rearrange("(k p) n -> p k n", p=128)
            )
            # K=128 zero-padded identity: rows 0-15 = I16, rest 0.
            ipad = cpool.tile([128, BL], bf16, tag="ipad")
            nc.gpsimd.memset(ipad[:], 0.0)
            make_identity(nc, ipad[:BL, :BL], nomemset=True)
            # K=128 zero-padded ones row (for bias injection matmuls).
            ones_pad = cpool.tile([128, 128], bf16, tag="ones_pad")
            nc.gpsimd.memset(ones_pad[:], 0.0)
            nc.gpsimd.memset(ones_pad[:1, :], 1.0)
            # combined bias (b_i2h + b_h2h), zero-padded to K=128 rows.
            bi = cpool.tile([1, D_H], f32, tag="bi")
            nc.sync.dma_start(bi[:], b_i2h.ap().unsqueeze(0))
            bh = cpool.tile([1, D_H], f32, tag="bh")
            nc.sync.dma_start(bh[:], b_h2h.ap().unsqueeze(0))
            bcomb = cpool.tile([128, D_H], bf16, tag="bcomb")
            nc.gpsimd.memset(bcomb[:], 0.0)
            nc.vector.tensor_add(bcomb[:1, :], bi[:], bh[:])
            # phase-3 bias (K=1 matmul; single mode switch is fine there)
            bo_f = cpool.tile([1, D_OUT], f32, tag="bo_f")
            nc.sync.dma_start(bo_f[:], b_out.ap().unsqueeze(0))
            bo = cpool.tile([1, D_OUT], bf16, tag="bo")
            nc.vector.tensor_copy(bo[:], bo_f[:])
            ones_row = cpool.tile([1, 128], bf16, tag="ones")
            nc.gpsimd.memset(ones_row[:], 1.0)

            # ---------------- Phase 1: xi' = x @ W_i2h + bcomb ----------------
            with (
                tc.tile_pool(name="p1", bufs=3) as p1pool,
                tc.tile_pool(name="p1ps", bufs=2, space="PSUM") as p1ps,
            ):
                for r in range(ROW_TILES):
                    b_idx = r // LW
                    lw = r % LW
                    xrow = p1pool.tile([128, D_IN], bf16, tag="xrow")
                    nc.gpsimd.dma_start(
                        xrow[:], x[128 * r : 128 * r + 128, :]
                    )
                    xT = p1pool.tile([128, KI, 128], bf16, tag="xT")
                    nc.sync.dma_start(xT[:], xrow[:], transpose=True)
                    xi_sb = p1pool.tile([128, D_H], bf16, tag="xi_sb")
                    for h in range(2):
                        ns = slice(512 * h, 512 * h + 512)
                        zp = p1ps.tile([128, 512], f32, tag="zp1")
                        for k in range(KI):
                            nc.tensor.matmul(
                                zp[:],
                                xT[:, k, :],
                                wi2h[:, k, ns],
                                start=(k == 0),
                                stop=False,
                            )
                        nc.tensor.matmul(
                            zp[:],
                            ones_pad[:],
                            bcomb[:, ns],
                            start=False,
                            stop=True,
                        )
                        nc.scalar.activation(xi_sb[:, ns], zp[:], AF.Copy)
                    # SWDGE write: keeps the HWDGE rings transpose-only, so
                    # Tile's DMA-transpose/DMA interlock never serializes the
                    # phase-1 pipeline.
                    nc.gpsimd.dma_start(
                        xi_dram[128 * lw : 128 * lw + 128, b_idx, :], xi_sb[:]
                    )

            # ---------------- Phase 2: recurrence ----------------
            with (
                tc.tile_pool(name="p2h", bufs=2) as hpool,
                tc.tile_pool(name="p2ps", bufs=2, space="PSUM") as p2ps,
            ):
                # xi ring: [p, cc, j, w] == plain [p, n]; rows 16-127 are
                # zeroed once (finite junk x zero ipad rows = exact 0).
                xi_bufs = [
                    cpool.tile(
                        [128, KH, NT, 32], bf16, tag=f"xi{i}", name=f"xi{i}"
                    )
                    for i in range(3)
                ]
                for t_ in xi_bufs:
                    nc.gpsimd.memset(t_[:], 0.0)
                # Two independent batch-8 recurrences ("groups") interleaved
                # on the PE: group B's matmul rounds execute during group A's
                # tanh+transpose tail and vice versa, so the PE never waits
                # on the serial ACT/DVE chain.
                GB = BL // 2
                # per-group transposed state, ping-pong, split lo/hi (k<4 /
                # k>=4) so next-step matmuls start as soon as the lo half
                # lands.
                hT = [
                    [
                        [
                            cpool.tile(
                                [128, 4, 32],
                                bf16,
                                tag=f"hT{g_}{p_}{h_}",
                                name=f"hT{g_}{p_}{h_}",
                            )
                            for h_ in range(2)
                        ]
                        for p_ in range(2)
                    ]
                    for g_ in range(2)
                ]
                for g_ in range(2):
                    for h_ in range(2):
                        nc.gpsimd.memset(hT[g_][0][h_][:], 0.0)
                # PSUM ping-pong tiles; zero once so the never-written junk
                # rows of each 32-row group stay finite for tanh.
                zeros_sb = cpool.tile([128, 256], f32, tag="zeros_sb")
                nc.gpsimd.memset(zeros_sb[:], 0.0)
                zps = [
                    [
                        p2ps.tile(
                            [128, 256], f32, tag=f"zp2_{g_}", name=f"zp2_{g_}{i}"
                        )
                        for i in range(2)
                    ]
                    for g_ in range(2)
                ]
                for zg in zps:
                    for z_ in zg:
                        nc.vector.tensor_copy(z_[:], zeros_sb[:])

                for t in range(l_steps):
                    xi_t = xi_bufs[t % 3]
                    nc.gpsimd.dma_start(
                        xi_t[:BL].rearrange("p a j w -> p (a j w)"),
                        xi_dram[t, :, :],
                    )
                    for g in range(2):
                        hT_cur = hT[g][t % 2]
                        hT_nxt = hT[g][(t + 1) % 2]
                        zp = zps[g][t % 2]
                        h_new = hpool.tile(
                            [128, KH, 32], bf16, tag=f"h_new{g}"
                        )
                        # xi inject: 4 col-tiled K=128 identity matmuls.
                        # ipad[:, 8g:8g+8] selects xi rows 8g..8g+8.
                        for j in range(NT):
                            nc.tensor.matmul(
                                zp[32 * j : 32 * j + GB, :],
                                ipad[:, GB * g : GB * g + GB],
                                xi_t[:, :, j, :],
                                start=True,
                                stop=False,
                                tile_position=(0, 32 * j),
                            )
                        # 8 k-rounds x 4 concurrent col-tiles
                        for k in range(KH):
                            lhsT = hT_cur[k // 4][:, k % 4, :GB]
                            for j in range(NT):
                                nc.tensor.matmul(
                                    zp[32 * j : 32 * j + GB, :],
                                    lhsT,
                                    whh[:, k, :, j, :],
                                    start=False,
                                    stop=(k == KH - 1),
                                    tile_position=(0, 32 * j),
                                )
                        # tanh + in-place 32x32 block transpose, split in
                        # halves so the lo transpose (gating next step's
                        # k<4 rounds) starts as early as possible.
                        for h_ in range(2):
                            hs = slice(4 * h_, 4 * h_ + 4)
                            nc.scalar.activation(
                                h_new[:, hs, :].rearrange("p a w -> p (a w)"),
                                zp[:, 128 * h_ : 128 * h_ + 128],
                                AF.Tanh,
                            )
                            nc.vector.transpose(
                                hT_nxt[h_][:].rearrange("p a w -> p (a w)"),
                                h_new[:, hs, :].rearrange("p a w -> p (a w)"),
                            )

                # ---------------- Phase 3: head ----------------
                # group A -> psum rows 0-7, group B -> rows 32-39 (col-tile
                # base partitions must be 32-aligned), then recombine.
                zp3 = p2ps.tile([128, D_OUT], f32, tag="zp3")
                for g in range(2):
                    base = 32 * g
                    hT_fin = hT[g][l_steps % 2]
                    nc.tensor.matmul(
                        zp3[base : base + GB, :],
                        ones_row[:, :GB],
                        bo[:],
                        start=True,
                        stop=False,
                        tile_position=(0, base),
                    )
                    for k in range(KH):
                        nc.tensor.matmul(
                            zp3[base : base + GB, :],
                            hT_fin[k // 4][:, k % 4, :GB],
                            wout[:, k, :],
                            start=False,
                            stop=(k == KH - 1),
                            tile_position=(0, base),
                        )
                out_sb = cpool.tile([128, D_OUT], f32, tag="out_sb")
                nc.vector.tensor_copy(out_sb[:GB], zp3[:GB])
                nc.vector.tensor_copy(
                    out_sb[32 : 32 + GB], zp3[32 : 32 + GB]
                )
                nc.sync.dma_start(out.ap()[:GB], out_sb[:GB])
                nc.sync.dma_start(out.ap()[GB:BL], out_sb[32 : 32 + GB])

    nc.compile()
    return nc


_CACHE = {}


def _get_nc(l_steps=L):
    if l_steps not in _CACHE:
        _CACHE[l_steps] = build_nc(l_steps)
    return _CACHE[l_steps]


def run(inputs, l_steps=L, trace=False, tmpdir=None):
    from concourse.bass_utils import run_bass_kernel_spmd

    nc = _get_nc(l_steps)
    x = np.asarray(inputs["x"], np.float32).reshape(B, L, D_IN)
    shared = {
        k: np.ascontiguousarray(np.asarray(inputs[k], np.float32))
        for k in ("W_i2h", "b_i2h", "W_h2h", "b_h2h", "W_out", "b_out")
    }
    in_maps = []
    for c in range(NCORES):
        m = dict(shared)
        m["x"] = np.ascontiguousarray(
            x[c * BL : (c + 1) * BL].reshape(BL * L, D_IN)
        )
        in_maps.append(m)
    res = run_bass_kernel_spmd(
        nc,
        in_maps,
        core_ids=list(range(NCORES)),
        trace=trace,
        tmpdir=tmpdir,
    )
    out = np.concatenate([r["out"] for r in res.results], axis=0)
    return out, res


def kernel(**inputs) -> np.ndarray:
    out, _ = run(inputs)
    return out


# revision 11
# speedup vs baseline: 3.2020x; 1.0005x over previous
"""Trainium2 Bass kernel for NaiveRNN.

Reference computation:
    xi = x @ W_i2h + b_i2h                      # [B, L, D_h]
    h_{t+1} = tanh(xi_t + h_t @ W_h2h + b_h2h)  # L sequential steps
    out = h_L @ W_out + b_out                   # [B, D_out]

Sharding: data-parallel over batch B=128 across 8 cores (16 rows each).
Weights replicated. No cross-core communication.

Per-core kernel design (all bf16 on the PE, f32 PSUM accumulation):

  Phase 1: xi' = x @ W_i2h + (b_i2h + b_h2h), written to DRAM scratch
      [L, B_loc, D_h] bf16. x tiles are cast-loaded bf16 (SWDGE) and
      transposed by HWDGE DMA-transpose (off the critical path, 3-deep
      pipelined). Bias injected via a zero-padded K=128 ones matmul so
      every matmul keeps the (128,128) array mode (no drains).

  Phase 2: 512 recurrence steps. The batch (16 rows) uses only 16 of the
      PE's 128 stationary columns, so the array is run in 128x32
      column-tiling mode: 4 concurrent matmuls per k-round, each
      streaming a different quarter of W_h2h's columns (N=256) ->
      ~4x less PE streaming time per step than a single-tile schedule.

      Column->tile assignment is interleaved at 32-column granularity
      (tile j takes dh columns with (dh//32)%4 == j, realized purely by
      strided access patterns on the W/xi streams: tiles laid out as
      [p, k, cc, j, w] are bit-identical to the plain layout). With this
      assignment the tanh output h_new[32j+b, 32cc+w] = h[b, 128cc+32j+w]
      turns into the next step's stationary operand hT[p, 32k+v] =
      h[v, 128k+p] via a single in-place 32x32 block transpose -- which
      is exactly what the DVE's stream-transpose instruction does.
      So the per-step recurrent path is:
        PE: 4 xi-inject matmuls (zero-padded identity, K=128) +
            8 rounds x 4 col-tiled matmuls (N=256)     -> PSUM [128,256]
        ACT: tanh PSUM -> SBUF bf16 [128,256]          (~0.4us)
        DVE: 2 stream-transposes [128,128]             (~0.3us each)
      No DMA and no PE-transpose on the critical path (the baseline lost
      ~5us/step to DMA-transpose latency + HAM cold-clock oscillation).

  Phase 3: out = h_L @ W_out + b_out (bf16 matmuls, f32 out).
"""

import numpy as np

B, L, D_IN, D_H, D_OUT = 128, 512, 512, 1024, 512
NCORES = 8
BL = B // NCORES            # 16 local batch rows
KI = D_IN // 128            # 4 k-chunks for input proj
KH = D_H // 128             # 8 k-chunks for recurrence
ROW_TILES = (BL * L) // 128  # 64 row tiles in phase 1
LW = L // 128               # l-windows per batch row (4)
NT = 4                      # column tiles in phase 2


def build_nc(l_steps=L):
    import concourse.bass as bass
    import concourse.mybir as mybir
    from concourse import bacc
    from concourse.tile import TileContext
    from concourse.masks import make_identity

    dt = mybir.dt
    f32, bf16 = dt.float32, dt.bfloat16
    AF = mybir.ActivationFunctionType

    nc = bacc.Bacc(
        "TRN2", target_bir_lowering=False, debug=False, num_devices=NCORES
    )
    x = nc.dram_tensor("x", [BL * L, D_IN], f32, kind="ExternalInput")
    W_i2h = nc.dram_tensor("W_i2h", [D_IN, D_H], f32, kind="ExternalInput")
    b_i2h = nc.dram_tensor("b_i2h", [D_H], f32, kind="ExternalInput")
    W_h2h = nc.dram_tensor("W_h2h", [D_H, D_H], f32, kind="ExternalInput")
    b_h2h = nc.dram_tensor("b_h2h", [D_H], f32, kind="ExternalInput")
    W_out = nc.dram_tensor("W_out", [D_H, D_OUT], f32, kind="ExternalInput")
    b_out = nc.dram_tensor("b_out", [D_OUT], f32, kind="ExternalInput")
    out = nc.dram_tensor("out", [BL, D_OUT], f32, kind="ExternalOutput")
    xi_dram = nc.dram_tensor("xi_scratch", [L, BL, D_H], bf16, kind="Internal")

    with TileContext(nc) as tc:
        with tc.tile_pool(name="const", bufs=1) as cpool:
            # Persistent weights/constants in SBUF; SWDGE casts f32->bf16.
            # whh laid out [p, k, cc, j, w] == plain [p, k, n] bit-for-bit;
            # slicing [:, k, :, j, :] yields the interleaved N=256 stream
            # for column-tile j.
            whh = cpool.tile([128, KH, KH, NT, 32], bf16, tag="whh")
            wi2h = cpool.tile([128, KI, D_H], bf16, tag="wi2h")
            wout = cpool.tile([128, KH, D_OUT], bf16, tag="wout")
            nc.gpsimd.dma_start(
                whh[:].rearrange("p k a j w -> p k (a j w)"),
                W_h2h.ap().rearrange("(k p) n -> p k n", p=128),
            )
            nc.gpsimd.dma_start(
                wi2h[:], W_i2h.ap().rearrange("(k p) n -> p k n", p=128)
            )
            nc.gpsimd.dma_start(
                wout[:], W_out.ap().rearrange("(k p) n -> p k n", p=128)
            )
            # K=128 zero-padded identity: rows 0-15 = I16, rest 0.
            ipad = cpool.tile([128, BL], bf16, tag="ipad")
            nc.gpsimd.memset(ipad[:], 0.0)
            make_identity(nc, ipad[:BL, :BL], nomemset=True)
            # K=128 zero-padded ones row (for bias injection matmuls).
            ones_pad = cpool.tile([128, 128], bf16, tag="ones_pad")
            nc.gpsimd.memset(ones_pad[:], 0.0)
            nc.gpsimd.memset(ones_pad[:1, :], 1.0)
            # combined bias (b_i2h + b_h2h), zero-padded to K=128 rows.
            bi = cpool.tile([1, D_H], f32, tag="bi")
            nc.sync.dma_start(bi[:], b_i2h.ap().unsqueeze(0))
            bh = cpool.tile([1, D_H], f32, tag="bh")
            nc.sync.dma_start(bh[:], b_h2h.ap().unsqueeze(0))
            bcomb = cpool.tile([128, D_H], bf16, tag="bcomb")
            nc.gpsimd.memset(bcomb[:], 0.0)
            nc.vector.tensor_add(bcomb[:1, :], bi[:], bh[:])
            # phase-3 bias (K=1 matmul; single mode switch is fine there)
            bo_f = cpool.tile([1, D_OUT], f32, tag="bo_f")
            nc.sync.dma_start(bo_f[:], b_out.ap().unsqueeze(0))
            bo = cpool.tile([1, D_OUT], bf16, tag="bo")
            nc.vector.tensor_copy(bo[:], bo_f[:])
            ones_row = cpool.tile([1, 128], bf16, tag="ones")
            nc.gpsimd.memset(ones_row[:], 1.0)

            # ---------------- Phase 1: xi' = x @ W_i2h + bcomb ----------------
            with (
                tc.tile_pool(name="p1", bufs=6) as p1pool,
                tc.tile_pool(name="p1ps", bufs=3, space="PSUM") as p1ps,
            ):
                for r in range(ROW_TILES):
                    b_idx = r // LW
                    lw = r % LW
                    xrow = p1pool.tile([128, D_IN], bf16, tag="xrow")
                    nc.gpsimd.dma_start(
                        xrow[:], x[128 * r : 128 * r + 128, :]
                    )
                    xT = p1pool.tile([128, KI, 128], bf16, tag="xT")
                    nc.sync.dma_start(xT[:], xrow[:], transpose=True)
                    xi_sb = p1pool.tile([128, D_H], bf16, tag="xi_sb")
                    for h in range(2):
                        ns = slice(512 * h, 512 * h + 512)
                        zp = p1ps.tile([128, 512], f32, tag="zp1")
                        for k in range(KI):
                            nc.tensor.matmul(
                                zp[:],
                                xT[:, k, :],
                                wi2h[:, k, ns],
                                start=(k == 0),
                                stop=False,
                            )
                        nc.tensor.matmul(
                            zp[:],
                            ones_pad[:],
                            bcomb[:, ns],
                            start=False,
                            stop=True,
                        )
                        nc.scalar.activation(xi_sb[:, ns], zp[:], AF.Copy)
                    # SWDGE write: keeps the HWDGE rings transpose-only, so
                    # Tile's DMA-transpose/DMA interlock never serializes the
                    # phase-1 pipeline.
                    nc.gpsimd.dma_start(
                        xi_dram[128 * lw : 128 * lw + 128, b_idx, :], xi_sb[:]
                    )

            # ---------------- Phase 2: recurrence ----------------
            with (
                tc.tile_pool(name="p2h", bufs=2) as hpool,
                tc.tile_pool(name="p2ps", bufs=2, space="PSUM") as p2ps,
            ):
                # xi ring: [p, cc, j, w] == plain [p, n]; rows 16-127 are
                # zeroed once (finite junk x zero ipad rows = exact 0).
                xi_bufs = [
                    cpool.tile(
                        [128, KH, NT, 32], bf16, tag=f"xi{i}", name=f"xi{i}"
                    )
                    for i in range(3)
                ]
                for t_ in xi_bufs:
                    nc.gpsimd.memset(t_[:], 0.0)
                # Two independent batch-8 recurrences ("groups") interleaved
                # on the PE: group B's matmul rounds execute during group A's
                # tanh+transpose tail and vice versa, so the PE never waits
                # on the serial ACT/DVE chain.
                GB = BL // 2
                # per-group transposed state, ping-pong, split lo/hi (k<4 /
                # k>=4) so next-step matmuls start as soon as the lo half
                # lands.
                hT = [
                    [
                        [
                            cpool.tile(
                                [128, 4, 32],
                                bf16,
                                tag=f"hT{g_}{p_}{h_}",
                                name=f"hT{g_}{p_}{h_}",
                            )
                            for h_ in range(2)
                        ]
                        for p_ in range(2)
                    ]
                    for g_ in range(2)
                ]
                for g_ in range(2):
                    for h_ in range(2):
                        nc.gpsimd.memset(hT[g_][0][h_][:], 0.0)
                # PSUM ping-pong tiles; zero once so the never-written junk
                # rows of each 32-row group stay finite for tanh.
                zeros_sb = cpool.tile([128, 256], f32, tag="zeros_sb")
                nc.gpsimd.memset(zeros_sb[:], 0.0)
                zps = [
                    [
                        p2ps.tile(
                            [128, 256], f32, tag=f"zp2_{g_}", name=f"zp2_{g_}{i}"
                        )
                        for i in range(2)
                    ]
                    for g_ in range(2)
                ]
                for zg in zps:
                    for z_ in zg:
                        nc.vector.tensor_copy(z_[:], zeros_sb[:])

                for t in range(l_steps):
                    xi_t = xi_bufs[t % 3]
                    nc.gpsimd.dma_start(
                        xi_t[:BL].rearrange("p a j w -> p (a j w)"),
                        xi_dram[t, :, :],
                    )
                    for g in range(2):
                        hT_cur = hT[g][t % 2]
                        hT_nxt = hT[g][(t + 1) % 2]
                        zp = zps[g][t % 2]
                        h_new = hpool.tile(
                            [128, KH, 32], bf16, tag=f"h_new{g}"
                        )
                        # xi inject: 4 col-tiled K=128 identity matmuls.
                        # ipad[:, 8g:8g+8] selects xi rows 8g..8g+8.
                        for j in range(NT):
                            nc.tensor.matmul(
                                zp[32 * j : 32 * j + GB, :],
                                ipad[:, GB * g : GB * g + GB],
                                xi_t[:, :, j, :],
                                start=True,
                                stop=False,
                                tile_position=(0, 32 * j),
                            )
                        # 8 k-rounds x 4 concurrent col-tiles
                        for k in range(KH):
                            lhsT = hT_cur[k // 4][:, k % 4, :GB]
                            for j in range(NT):
                                nc.tensor.matmul(
                                    zp[32 * j : 32 * j + GB, :],
                                    lhsT,
                                    whh[:, k, :, j, :],
                                    start=False,
                                    stop=(k == KH - 1),
                                    tile_position=(0, 32 * j),
                                )
                        # tanh + in-place 32x32 block transpose, split in
                        # halves so the lo transpose (gating next step's
                        # k<4 rounds) starts as early as possible.
                        for h_ in range(2):
                            hs = slice(4 * h_, 4 * h_ + 4)
                            nc.scalar.activation(
                                h_new[:, hs, :].rearrange("p a w -> p (a w)"),
                                zp[:, 128 * h_ : 128 * h_ + 128],
                                AF.Tanh,
                            )
                            nc.vector.transpose(
                                hT_nxt[h_][:].rearrange("p a w -> p (a w)"),
                                h_new[:, hs, :].rearrange("p a w -> p (a w)"),
                            )

                # ---------------- Phase 3: head ----------------
                # group A -> psum rows 0-7, group B -> rows 32-39 (col-tile
                # base partitions must be 32-aligned), then recombine.
                zp3 = p2ps.tile([128, D_OUT], f32, tag="zp3")
                for g in range(2):
                    base = 32 * g
                    hT_fin = hT[g][l_steps % 2]
                    nc.tensor.matmul(
                        zp3[base : base + GB, :],
                        ones_row[:, :GB],
                        bo[:],
                        start=True,
                        stop=False,
                        tile_position=(0, base),
                    )
                    for k in range(KH):
                        nc.tensor.matmul(
                            zp3[base : base + GB, :],
                            hT_fin[k // 4][:, k % 4, :GB],
                            wout[:, k, :],
                            start=False,
                            stop=(k == KH - 1),
                            tile_position=(0, base),
                        )
                out_sb = cpool.tile([128, D_OUT], f32, tag="out_sb")
                nc.vector.tensor_copy(out_sb[:GB], zp3[:GB])
                nc.vector.tensor_copy(
                    out_sb[32 : 32 + GB], zp3[32 : 32 + GB]
                )
                nc.sync.dma_start(out.ap()[:GB], out_sb[:GB])
                nc.sync.dma_start(out.ap()[GB:BL], out_sb[32 : 32 + GB])

    nc.compile()
    return nc


_CACHE = {}


def _get_nc(l_steps=L):
    if l_steps not in _CACHE:
        _CACHE[l_steps] = build_nc(l_steps)
    return _CACHE[l_steps]


def run(inputs, l_steps=L, trace=False, tmpdir=None):
    from concourse.bass_utils import run_bass_kernel_spmd

    nc = _get_nc(l_steps)
    x = np.asarray(inputs["x"], np.float32).reshape(B, L, D_IN)
    shared = {
        k: np.ascontiguousarray(np.asarray(inputs[k], np.float32))
        for k in ("W_i2h", "b_i2h", "W_h2h", "b_h2h", "W_out", "b_out")
    }
    in_maps = []
    for c in range(NCORES):
        m = dict(shared)
        m["x"] = np.ascontiguousarray(
            x[c * BL : (c + 1) * BL].reshape(BL * L, D_IN)
        )
        in_maps.append(m)
    res = run_bass_kernel_spmd(
        nc,
        in_maps,
        core_ids=list(range(NCORES)),
        trace=trace,
        tmpdir=tmpdir,
    )
    out = np.concatenate([r["out"] for r in res.results], axis=0)
    return out, res


def kernel(**inputs) -> np.ndarray:
    out, _ = run(inputs)
    return out
